# revision 1
# baseline (speedup 1.0000x reference)
"""Trainium2 Bass kernel for CausalWanSelfAttention (frame-causal windowed
attention with QK-RMSNorm + RoPE), sharded over 8 NeuronCores.

Sharding: each core owns T = (h*w)/8 tokens of every frame (frame-balanced
interleave).  Each core computes Q/K/V projections + RMSNorm + RoPE for its
own tokens, K/V are exchanged with two AllGathers, attention + O-projection
are computed locally for the core's query tokens.

Device layouts:
  - q/k feature-major [ch, tok] (channels on partitions), with each head's
    128 channels permuted to [re(0..63) | im(0..63)] so RoPE works on
    contiguous partition blocks (permutation is folded into Wq/Wk on host).
  - v token-major [tok, ch] (natural channel order).
  - scores computed as s^T [keys, q]  ->  softmax denominator via
    ones-matmul (partition reduction on the tensor engine), exp on ACT.
  - attention out o^T [ch, tok]; per-head 1/Z applied by DVE during PSUM
    eviction with a partition-broadcast tile.
  - RMSNorm scale r (per token) is folded into the RoPE cos/sin tables
    (scalar multiplication commutes with rotation); per-channel gain g and
    bias b are folded into the ACT eviction (per-partition scale/bias).
"""

import math
import sys
from contextlib import ExitStack

import numpy as np

if "/opt/trn_rl_repo" not in sys.path:
    sys.path.insert(0, "/opt/trn_rl_repo")

import ml_dtypes

BF16 = ml_dtypes.bfloat16
NC = 8  # cores
D = 128  # head dim
EPS = 1e-6


# ---------------------------------------------------------------------------
# helpers
# ---------------------------------------------------------------------------
def _pieces(lo, hi, T):
    """Split the global (within-frame) token range [lo, hi) into per-core
    pieces.  Returns [(core, a, b)] with a/b local to the core's frame-chunk."""
    out = []
    c = lo // T
    while lo < hi:
        b = min(hi, (c + 1) * T)
        out.append((c, lo - c * T, b - c * T))
        lo = b
        c += 1
    return out


def _segs(q0, S, bank=512):
    """Split [q0, S) at multiples of `bank` -> list of absolute (qa, qb)."""
    pts = [q0]
    nxt = (q0 // bank + 1) * bank
    while nxt < S:
        pts.append(nxt)
        nxt += bank
    pts.append(S)
    return [(pts[i], pts[i + 1]) for i in range(len(pts) - 1)]


def _chunks(frame_len, width=128):
    return [(g * width, min(frame_len, (g + 1) * width))
            for g in range((frame_len + width - 1) // width)]


# ---------------------------------------------------------------------------
# device program
# ---------------------------------------------------------------------------
_BUILD_CACHE = {}


def build_program(NH, F, T, allowed_kf, cap_waits=True):
    """Build the SPMD Bass program (identical on all 8 cores).

    NH: number of heads; F: frames; T: tokens per (core, frame);
    allowed_kf[qf] = list of key frames query-frame qf may attend to
    (must make, for each kf, the attending q-set a contiguous suffix of
    frames -- true for causal masks).
    """
    key = (NH, F, T, tuple(tuple(a) for a in allowed_kf), cap_waits)
    if key in _BUILD_CACHE:
        return _BUILD_CACHE[key]

    import concourse.bass as bass
    import concourse.mybir as mybir
    import concourse.tile as tile
    from concourse.mybir import ActivationFunctionType as AF

    dt = mybir.dt
    DIM = NH * D
    S = F * T              # tokens per core
    FRAME = NC * T         # tokens per frame
    NHALF = 2
    H0 = (S + 1) // 2      # token halves for the q/k projections
    SLICE = min(512, DIM)  # out-channel slice for v/o projections
    NSL = DIM // SLICE
    TOKCH = _chunks(S, 128)  # token chunks for v/o projections

    # for each key frame kf: the first query frame that attends to it, and
    # check the q-set is a suffix
    first_qf = {}
    for kf in range(F):
        qs = [qf for qf in range(F) if kf in allowed_kf[qf]]
        assert qs, f"key frame {kf} unused"
        assert qs == list(range(qs[0], F)), "non-suffix q-set unsupported"
        first_qf[kf] = qs[0]

    nc = bass.Bass()

    # ---------------- I/O ----------------
    xT_d = nc.dram_tensor("xT", [DIM, S], dt.bfloat16, kind="ExternalInput")
    w_d = {}
    for nm in ("wqT", "wkT", "wvT", "woT"):
        w_d[nm] = nc.dram_tensor(nm, [DIM, DIM], dt.bfloat16, kind="ExternalInput")
    # packed per-channel affine constants: bq|gq|bq*gq|bk|gk|bk*gk
    bias_d = nc.dram_tensor("bias_pack", [128, 6 * NH], dt.float32,
                            kind="ExternalInput")
    bv_d = nc.dram_tensor("bv_r", [1, DIM], dt.bfloat16, kind="ExternalInput")
    bo_d = nc.dram_tensor("bo_r", [1, DIM], dt.float32, kind="ExternalInput")
    angS_d = nc.dram_tensor("angS", [128, S], dt.float32, kind="ExternalInput")
    angC_d = nc.dram_tensor("angC", [128, S], dt.float32, kind="ExternalInput")
    out_d = nc.dram_tensor("out", [S, DIM], dt.float32, kind="ExternalOutput")

    rg = [list(range(NC))]
    inv_sqrt_d = 1.0 / math.sqrt(D)

    with tile.TileContext(nc) as tc, ExitStack() as ctx:
        dram = ctx.enter_context(tc.tile_pool(name="dram", bufs=1, space="DRAM"))
        k_loc = dram.tile([DIM, S], dt.bfloat16)
        v_loc = dram.tile([S, DIM], dt.bfloat16)
        k_all = dram.tile([NC * DIM, S], dt.bfloat16, addr_space="Shared")
        v_all = dram.tile([NC * S, DIM], dt.bfloat16, addr_space="Shared")

        const = ctx.enter_context(tc.tile_pool(name="const", bufs=1))
        resid = ctx.enter_context(tc.tile_pool(name="resid", bufs=1))

        ones_key = const.tile([128, 1], dt.bfloat16)
        nc.vector.memset(ones_key, 1.0)
        ones_row = const.tile([1, 128], dt.bfloat16)
        nc.vector.memset(ones_row, 1.0)
        negpi = const.tile([128, 1], dt.float32)
        nc.vector.memset(negpi, -math.pi)
        eps_t = const.tile([128, 1], dt.float32)
        nc.vector.memset(eps_t, EPS)

        # constant / bias tiles (one DMA for the packed affine constants)
        bias_sb = const.tile([128, 6 * NH], dt.float32)
        nc.sync.dma_start(out=bias_sb[:], in_=bias_d[:])
        bq_sb = bias_sb[:, 0 * NH:1 * NH]
        gq_sb = bias_sb[:, 1 * NH:2 * NH]
        bqgq_sb = bias_sb[:, 2 * NH:3 * NH]
        bk_sb = bias_sb[:, 3 * NH:4 * NH]
        gk_sb = bias_sb[:, 4 * NH:5 * NH]
        bkgk_sb = bias_sb[:, 5 * NH:6 * NH]
        bv_sb = const.tile([1, DIM], dt.bfloat16)
        nc.sync.dma_start(out=bv_sb[:], in_=bv_d[:])
        bo_bc = const.tile([128, DIM], dt.float32)
        nc.sync.dma_start(
            out=bo_bc[:],
            in_=bass.AP(tensor=bo_d[:].tensor, offset=bo_d[:].offset,
                        ap=[[0, 128]] + bo_d[:].ap[1:]),
        )

        # x (feature-major), resident
        xT_sb = resid.tile([128, NH, S], dt.bfloat16)
        nc.sync.dma_start(out=xT_sb[:], in_=xT_d[:].rearrange("(m p) s -> p m s", p=128))

        # raw RoPE sin/cos (shared q/k)
        angS_sb = resid.tile([128, S], dt.float32)
        angC_sb = resid.tile([128, S], dt.float32)
        nc.sync.dma_start(out=angS_sb[:], in_=angS_d[:])
        nc.sync.dma_start(out=angC_sb[:], in_=angC_d[:])
        # angles arrive host-canonicalized to [-pi, pi] (ACT Sin table range)
        sin_raw = resid.tile([128, S], dt.float32)
        cos_raw = resid.tile([128, S], dt.float32)
        nc.scalar.activation(sin_raw[:], angS_sb[:], AF.Sin)
        nc.scalar.activation(cos_raw[:], angC_sb[:], AF.Sin)

        qhat = resid.tile([128, NH, S], dt.bfloat16)
        khat = resid.tile([128, NH, S], dt.bfloat16)
        qrot = resid.tile([128, NH, S], dt.bfloat16)
        krot = resid.tile([128, NH, S], dt.bfloat16)
        r_q = resid.tile([1, S], dt.float32)
        r_k = resid.tile([1, S], dt.float32)
        oT_sb = resid.tile([128, NH, S], dt.bfloat16)

        halves = [(0, H0), (H0, S)] if S > H0 else [(0, S)]

        # ---------------- Q/K projections + RMS stats ----------------
        def qk_proj(wname, bias_sb, gain_sb, bg_sb, hat, r_sb):
          with ExitStack() as pctx:
            wpool = pctx.enter_context(tc.tile_pool(name=f"w_{wname}", bufs=1))
            pspool = pctx.enter_context(
                tc.tile_pool(name=f"ps_{wname}", bufs=4, space="PSUM"))
            sspool = pctx.enter_context(
                tc.tile_pool(name=f"ss_{wname}", bufs=2, space="PSUM"))
            evpool = pctx.enter_context(tc.tile_pool(name=f"ev_{wname}", bufs=3))
            w_sb = wpool.tile([128, NH, DIM], dt.bfloat16, name=f"wsb_{wname}")
            nc.sync.dma_start(
                out=w_sb[:], in_=w_d[wname][:].rearrange("(kc p) n -> p kc n", p=128))
            ss_ps = {}
            for hi, (ha, hb) in enumerate(halves):
                ss_ps[hi] = sspool.tile([1, hb - ha], dt.float32, tag="ss", name=f"ss{hi}")
            for m in range(NH):
                ps = {}
                for hi, (ha, hb) in enumerate(halves):
                    ps[hi] = pspool.tile([128, hb - ha], dt.float32, tag="ps", name=f"ps{hi}")
                for kc in range(NH):
                    for hi, (ha, hb) in enumerate(halves):
                        nc.tensor.matmul(ps[hi][:, :hb - ha],
                                         w_sb[:, kc, m * 128:(m + 1) * 128],
                                         xT_sb[:, kc, ha:hb],
                                         start=(kc == 0), stop=(kc == NH - 1))
                for hi, (ha, hb) in enumerate(halves):
                    hw_ = hb - ha
                    sq = evpool.tile([128, H0], dt.bfloat16, tag="sq")
                    # (q + b)^2
                    nc.scalar.activation(sq[:, :hw_], ps[hi][:, :hw_], AF.Square,
                                         bias=bias_sb[:, m:m + 1])
                    # qhat = (q + b) * g = q*g + b*g
                    nc.scalar.activation(hat[:, m, ha:hb], ps[hi][:, :hw_],
                                         AF.Identity, bias=bg_sb[:, m:m + 1],
                                         scale=gain_sb[:, m:m + 1])
                    nc.tensor.matmul(ss_ps[hi][0:1, :hw_], ones_key[:],
                                     sq[:, :hw_],
                                     start=(m == 0), stop=(m == NH - 1))
            for hi, (ha, hb) in enumerate(halves):
                hw_ = hb - ha
                rt = evpool.tile([1, H0], dt.float32, tag="rt")
                # sqrt(mean(q^2) + eps)
                nc.scalar.activation(rt[0:1, :hw_], ss_ps[hi][0:1, :hw_], AF.Sqrt,
                                     bias=eps_t[0:1, :], scale=1.0 / DIM)
                nc.vector.reciprocal(r_sb[0:1, ha:hb], rt[0:1, :hw_])

        # ---------------- RoPE ----------------
        def rope(hat, rot, r_sb, tag):
          with ExitStack() as pctx:
            rp = pctx.enter_context(tc.tile_pool(name=f"rope_{tag}", bufs=3))
            r_dram = dram.tile([1, S], dt.float32, name=f"rdram_{tag}")
            nc.sync.dma_start(out=r_dram[:], in_=r_sb[0:1, :])
            rb = resid.tile([128, S], dt.float32, name=f"rb_{tag}")
            nc.sync.dma_start(
                out=rb[:],
                in_=bass.AP(tensor=r_dram.tensor, offset=r_dram[0:1, :].offset,
                            ap=[[0, 128]] + r_dram[0:1, :].ap[1:]))
            ct = resid.tile([128, S], dt.bfloat16, name=f"cos_{tag}")
            st = resid.tile([128, S], dt.bfloat16, name=f"sin_{tag}")
            nc.vector.tensor_mul(ct[:], cos_raw[:], rb[:])
            nc.vector.tensor_mul(st[:], sin_raw[:], rb[:])
            for m in range(NH):
                sw = rp.tile([128, S], dt.bfloat16, tag="sw")
                nc.sync.dma_start(out=sw[0:64, :], in_=hat[64:128, m, :])
                nc.sync.dma_start(out=sw[64:128, :], in_=hat[0:64, m, :])
                t1 = rp.tile([128, S], dt.bfloat16, tag="t1")
                t2 = rp.tile([128, S], dt.bfloat16, tag="t2")
                nc.vector.tensor_mul(t1[:], hat[:, m, :], ct[:])
                nc.vector.tensor_mul(t2[:], sw[:], st[:])
                nc.vector.tensor_add(rot[:, m, :], t1[:], t2[:])

        # ---------------- V projection (token-major) ----------------
        def v_proj():
          with ExitStack() as pctx:
            wpool = pctx.enter_context(tc.tile_pool(name="w_v", bufs=1))
            pspool = pctx.enter_context(
                tc.tile_pool(name="ps_v", bufs=len(TOKCH) + 1, space="PSUM"))
            evpool = pctx.enter_context(tc.tile_pool(name="ev_v", bufs=3))
            w_sb = wpool.tile([128, NH, DIM], dt.bfloat16, name="wsb_v")
            nc.sync.dma_start(
                out=w_sb[:], in_=w_d["wvT"][:].rearrange("(kc p) n -> p kc n", p=128))
            for sl in range(NSL):
                ps = {}
                for ti in range(len(TOKCH)):
                    ps[ti] = pspool.tile([128, SLICE], dt.float32, tag="vps", name=f"vps{ti}")
                for kc in range(NH):
                    for ti, (ta, tb) in enumerate(TOKCH):
                        nc.tensor.matmul(ps[ti][:tb - ta, :], xT_sb[:, kc, ta:tb],
                                         w_sb[:, kc, sl * SLICE:(sl + 1) * SLICE],
                                         start=(kc == 0), stop=False)
                for ti, (ta, tb) in enumerate(TOKCH):
                    tw = tb - ta
                    nc.tensor.matmul(ps[ti][:tw, :], ones_row[0:1, :tw],
                                     bv_sb[0:1, sl * SLICE:(sl + 1) * SLICE],
                                     start=False, stop=True)
                    vt = evpool.tile([128, SLICE], dt.bfloat16, tag="vev")
                    nc.scalar.activation(vt[:tw, :], ps[ti][:tw, :], AF.Copy)
                    nc.sync.dma_start(
                        out=v_loc[ta:tb, sl * SLICE:(sl + 1) * SLICE],
                        in_=vt[:tw, :])

        # ---- phase order: K first (collective early), then V, then Q ----
        qk_proj("wkT", bk_sb, gk_sb, bkgk_sb, khat, r_k)
        rope(khat, krot, r_k, "k")
        for m in range(NH):
            nc.sync.dma_start(out=k_loc[m * 128:(m + 1) * 128, :], in_=krot[:, m, :])
        v_proj()
        qk_proj("wqT", bq_sb, gq_sb, bqgq_sb, qhat, r_q)
        rope(qhat, qrot, r_q, "q")
        # collectives last: the tile scheduler keeps program-later instructions
        # behind collectives, so everything not needing k_all/v_all stays ahead
        nc.gpsimd.collective_compute(
            "AllGather", mybir.AluOpType.bypass, ins=[k_loc[:]], outs=[k_all[:]],
            replica_groups=rg)
        nc.gpsimd.collective_compute(
            "AllGather", mybir.AluOpType.bypass, ins=[v_loc[:]], outs=[v_all[:]],
            replica_groups=rg)

        # ---------------- attention ----------------
        # Head pairs (one K DMA covers 2 heads); per-(pair, core) loads cover
        # all F frames in one DMA.  Score tiles span 2 PSUM banks so exp is a
        # single ACT instruction per key-chunk.  The softmax denominator Z is
        # accumulated by ones-matmuls into spare columns of the o-seg1 bank.
        actx = ctx.enter_context(ExitStack())
        att_k = actx.enter_context(tc.tile_pool(name="att_k", bufs=3))
        att_v = actx.enter_context(tc.tile_pool(name="att_v", bufs=3))
        att_s = actx.enter_context(tc.tile_pool(name="att_s", bufs=2, space="PSUM"))
        att_o = actx.enter_context(tc.tile_pool(name="att_o", bufs=1, space="PSUM"))
        att_p = actx.enter_context(tc.tile_pool(name="att_p", bufs=6))
        att_m = actx.enter_context(tc.tile_pool(name="att_m", bufs=2))

        oseg = _segs(0, S)  # absolute segments for o accumulation
        assert len(oseg) <= 2
        KCH = _chunks(T, 128)  # key chunks within one core's frame slice
        z_in_o = False
        w1 = 0
        ZB = [0] + [b for (a, b) in oseg]
        att_z = actx.enter_context(
            tc.tile_pool(name="att_z", bufs=1, space="PSUM"))
        NZR = len(ZB) - 1

        work = []
        for c in range(NC):
            for kf in range(F):
                work.append((c, kf))

        def seg_isect(q0):
            out = []
            for (oa, ob) in oseg:
                qa = max(q0, oa)
                if qa < ob:
                    out.append((qa, ob))
            return out

        si_first = {}; si_last = {}
        zr_first = {}; zr_last = {}
        for wi, (c, kf) in enumerate(work):
            q0 = T * first_qf[kf]
            for si, (oa, ob) in enumerate(oseg):
                if max(q0, oa) < ob:
                    si_first.setdefault(si, wi); si_last[si] = wi
            for r in range(NZR):
                if max(q0, ZB[r]) < ZB[r + 1]:
                    zr_first.setdefault(r, wi); zr_last[r] = wi

        NPG = NH // 2
        for pg in range(NPG):
            o_ps = {}
            z_ps = {}
            for hi in range(2):
                for si, (qa, qb) in enumerate(oseg):
                    o_ps[(hi, si)] = att_o.tile([128, qb - qa], dt.float32,
                                                tag=f"o{hi}{si}", name=f"o{hi}{si}")
                z_ps[hi] = att_z.tile([128, 512], dt.float32, tag=f"z{hi}",
                                      name=f"z{hi}")

            def z_target(hi, r, qa, qb):
                # z region r lives at partition 32*r of the head's z bank
                return z_ps[hi][32 * r:32 * r + 1, qa - ZB[r]:qb - ZB[r]]

            # number of z matmuls per head (to place the group-closing stop)
            nztot = 0
            for (c, kf) in work:
                q0 = T * first_qf[kf]
                for (qa, qb) in seg_isect(q0):
                    for r in range(NZR):
                        if max(qa, ZB[r]) < min(qb, ZB[r + 1]):
                            nztot += len(KCH)
            z_idx = {0: 0, 1: 0}

            for wi, (c, kf) in enumerate(work):
                if kf == 0:
                    kr_t = att_k.tile([128, 2, S], dt.bfloat16, tag="kr")
                    nc.sync.dma_start(
                        out=kr_t[:],
                        in_=k_all[c * DIM + pg * 256:c * DIM + (pg + 1) * 256, :]
                        .rearrange("(hi p) t -> p hi t", p=128))
                    v_t = {}
                    v_view = v_all[c * S:(c + 1) * S, pg * 256:(pg + 1) * 256] \
                        .rearrange("(kf t) c2 -> t kf c2", kf=F)
                    for ti, (ta, tb) in enumerate(KCH):
                        v_t[ti] = att_v.tile([128, F, 256], dt.bfloat16,
                                             tag=f"v{ti}", name=f"v{ti}")
                        nc.sync.dma_start(out=v_t[ti][:tb - ta, :, :],
                                          in_=v_view[ta:tb])
                q0 = T * first_qf[kf]
                segs = seg_isect(q0)
                if not segs:
                    continue
                sega, segb = segs[0][0], segs[-1][1]
                qw_full = segb - sega
                for ti, (ta, tb) in enumerate(KCH):
                    kw = tb - ta
                    for hi in range(2):
                        for (qa, qb) in segs:
                            qw = qb - qa
                            s_t = att_s.tile([128, 512], dt.float32, tag="s")
                            nc.tensor.matmul(s_t[:kw, :qw],
                                             kr_t[:, hi, kf * T + ta:kf * T + tb],
                                             qrot[:, 2 * pg + hi, qa:qb],
                                             start=True, stop=True)
                            p_t = att_p.tile([128, 512], dt.bfloat16, tag="p")
                            nc.scalar.activation(p_t[:kw, :qw], s_t[:kw, :qw],
                                                 AF.Exp, scale=inv_sqrt_d)
                            for si, (oa, ob) in enumerate(oseg):
                                ia, ib = max(qa, oa), min(qb, ob)
                                if ia >= ib:
                                    continue
                                first = si_first[si] == wi and ti == 0
                                last = si_last[si] == wi and ti == len(KCH) - 1
                                if z_in_o and si == len(oseg) - 1:
                                    last = False  # z closes this bank's group
                                nc.tensor.matmul(
                                    o_ps[(hi, si)][:, ia - oa:ib - oa],
                                    v_t[ti][:kw, kf, hi * 128:(hi + 1) * 128],
                                    p_t[:kw, ia - qa:ib - qa],
                                    start=first, stop=last)
                            for r in range(NZR):
                                ia, ib = max(qa, ZB[r]), min(qb, ZB[r + 1])
                                if ia >= ib:
                                    continue
                                first = zr_first[r] == wi and ti == 0
                                last = zr_last[r] == wi and ti == len(KCH) - 1
                                nc.tensor.matmul(
                                    z_target(hi, r, ia, ib), ones_key[:kw, :],
                                    p_t[:kw, ia - qa:ib - qa],
                                    start=first, stop=last)
            # 1/Z and eviction per head
            for hi in range(2):
                hh = 2 * pg + hi
                z_sb = att_m.tile([128, 512], dt.float32, tag="zsb", name="zsb")
                z_dram = dram.tile([1, S], dt.float32, tag="zdram", bufs=2,
                                   name="zdram")
                for r in range(NZR):
                    wr = ZB[r + 1] - ZB[r]
                    zt = z_target(hi, r, ZB[r], ZB[r + 1])
                    p0 = zt.base_partition()
                    nc.scalar.activation(z_sb[p0:p0 + 1, :wr], zt, AF.Copy)
                    nc.vector.reciprocal(z_sb[p0:p0 + 1, :wr],
                                         z_sb[p0:p0 + 1, :wr])
                    nc.sync.dma_start(out=z_dram[0:1, ZB[r]:ZB[r + 1]],
                                      in_=z_sb[p0:p0 + 1, :wr])
                izb = att_m.tile([128, S], dt.float32, tag="izb", name="izb")
                nc.sync.dma_start(
                    out=izb[:],
                    in_=bass.AP(tensor=z_dram.tensor, offset=z_dram[0:1, :].offset,
                                ap=[[0, 128]] + z_dram[0:1, :].ap[1:]))
                for si, (oa, ob) in enumerate(oseg):
                    nc.vector.tensor_mul(oT_sb[:, hh, oa:ob],
                                         o_ps[(hi, si)][:, :ob - oa],
                                         izb[:, oa:ob])

        actx.close()  # release attention PSUM banks before the O-projection

        # ---------------- O projection ----------------
        wpool = ctx.enter_context(tc.tile_pool(name="w_o", bufs=3))
        pspool = ctx.enter_context(
            tc.tile_pool(name="ps_o", bufs=len(TOKCH) + 1, space="PSUM"))
        evpool = ctx.enter_context(tc.tile_pool(name="ev_o", bufs=3))
        for sl in range(NSL):
            ps = {}
            for ti in range(len(TOKCH)):
                ps[ti] = pspool.tile([128, SLICE], dt.float32, tag="ops", name=f"ops{ti}")
            for m in range(NH):
                wt = wpool.tile([128, SLICE], dt.bfloat16, tag="wo")
                nc.sync.dma_start(
                    out=wt[:],
                    in_=w_d["woT"][m * 128:(m + 1) * 128,
                                   sl * SLICE:(sl + 1) * SLICE])
                for ti, (ta, tb) in enumerate(TOKCH):
                    nc.tensor.matmul(ps[ti][:tb - ta, :], oT_sb[:, m, ta:tb],
                                     wt[:], start=(m == 0), stop=(m == NH - 1))
            for ti, (ta, tb) in enumerate(TOKCH):
                tw = tb - ta
                ot = evpool.tile([128, SLICE], dt.float32, tag="oev")
                nc.vector.tensor_add(ot[:tw, :], ps[ti][:tw, :],
                                     bo_bc[:tw, sl * SLICE:(sl + 1) * SLICE])
                nc.sync.dma_start(
                    out=out_d[ta:tb, sl * SLICE:(sl + 1) * SLICE],
                    in_=ot[:tw, :])

    if cap_waits:
        _cap_sync_waits(nc, mybir)
    _BUILD_CACHE[key] = nc
    return nc


def _cap_sync_waits(nc, mybir, cap=1):
    """Walrus engine-instruction structs only have a limited number of sync
    wait slots.  Hoist excess waits onto InstNoOp carriers placed immediately
    before the instruction on the same engine stream."""
    exempt = (mybir.InstNoOp, mybir.InstEventSemaphore,
              mybir.InstAllEngineBarrier)
    for f in nc.m.functions:
        for bb in f.blocks:
            out = []
            changed = False
            for inst in bb.instructions:
                si = inst.sync_info
                if (si is None or len(si.on_wait) <= cap
                        or isinstance(inst, exempt)):
                    out.append(inst)
                    continue
                waits = list(si.on_wait)
                keep, excess = waits[:cap], waits[cap:]
                while excess:
                    batch, excess = excess[:cap], excess[cap:]
                    out.append(mybir.InstNoOp(
                        name=f"{inst.name}-w{len(out)}",
                        engine=inst.engine,
                        bass_nofuse=True,
                        sync_info=mybir.SyncInfo(on_wait=batch, on_update=[]),
                    ))
                inst.sync_info = mybir.SyncInfo(on_wait=keep,
                                                on_update=list(si.on_update))
                out.append(inst)
                changed = True
            if changed:
                bb.instructions = out


# ---------------------------------------------------------------------------
# host side
# ---------------------------------------------------------------------------
def _perm(NH):
    p = np.empty(NH * D, np.int64)
    for hh in range(NH):
        base = hh * D
        for j in range(D // 2):
            p[base + j] = base + 2 * j
            p[base + D // 2 + j] = base + 2 * j + 1
    return p


def _host_inputs(x, freqs, Wq, bq, Wk, bk, Wv, bv, Wo, bo, gq, gk,
                 f, h, w, num_heads, local_attn_size, sink_size, start_frame):
    NH = num_heads
    DIM = NH * D
    FRAME = h * w
    assert FRAME % NC == 0
    T = FRAME // NC
    S = f * T
    perm = _perm(NH)

    def bf(a):
        return np.ascontiguousarray(a, dtype=np.float32).astype(BF16)

    wqT = bf(Wq[perm].T)
    wkT = bf(Wk[perm].T)
    wvT = bf(Wv.T)
    woT = bf(Wo.T)
    def chunkmajor(a):
        return np.asarray(a, np.float32)[perm].reshape(NH, D).T
    bias_pack = np.ascontiguousarray(np.concatenate(
        [chunkmajor(bq), chunkmajor(gq), chunkmajor(bq) * chunkmajor(gq),
         chunkmajor(bk), chunkmajor(gk), chunkmajor(bk) * chunkmajor(gk)],
        axis=1), np.float32)
    bv_r = bf(bv.reshape(1, DIM))
    bo_r = np.ascontiguousarray(bo.reshape(1, DIM), np.float32)

    c = D // 2
    c1 = c // 3
    c0 = c - 2 * c1
    freqs = np.asarray(freqs, np.float32)

    in_maps = []
    tok_idx = []
    for core in range(NC):
        idx = np.concatenate(
            [fr * FRAME + T * core + np.arange(T) for fr in range(f)])
        tok_idx.append(idx)
        xT = bf(np.asarray(x[0], np.float32)[idx].T)
        fr = idx // FRAME
        rem = idx % FRAME
        hh_i = rem // w
        ww_i = rem % w
        ang = np.empty((c, S), np.float32)
        ang[:c0, :] = freqs[start_frame + fr][:, :c0].T
        ang[c0:c0 + c1, :] = freqs[hh_i][:, c0:c0 + c1].T
        ang[c0 + c1:, :] = freqs[ww_i][:, c0 + c1:c].T
        def wrap(a):
            a = np.asarray(a, np.float64)
            return (a - 2 * np.pi * np.round(a / (2 * np.pi))).astype(np.float32)
        # top half encodes -sin via the (ang + pi) phase shift
        angS = np.ascontiguousarray(
            np.concatenate([wrap(ang + np.pi), wrap(ang)], 0), np.float32)
        angC = np.ascontiguousarray(
            np.concatenate([wrap(ang + np.pi / 2), wrap(ang + np.pi / 2)], 0),
            np.float32)
        in_maps.append({
            "xT": xT, "wqT": wqT, "wkT": wkT, "wvT": wvT, "woT": woT,
            "bias_pack": bias_pack,
            "bv_r": bv_r, "bo_r": bo_r, "angS": angS, "angC": angC,
        })
    return in_maps, tok_idx, T, S


def _allowed(f, local_attn_size, sink_size):
    return [
        [kf for kf in range(f)
         if kf <= qf and (qf - kf < local_attn_size or kf < sink_size)]
        for qf in range(f)
    ]


def kernel(x, freqs, Wq, bq, Wk, bk, Wv, bv, Wo, bo, gq, gk,
           f, h, w, num_heads, local_attn_size, sink_size, start_frame,
           _trace=False):
    from concourse.bass_utils import run_bass_kernel_spmd

    f = int(f); h = int(h); w = int(w)
    num_heads = int(num_heads)
    local_attn_size = int(local_attn_size)
    sink_size = int(sink_size)
    start_frame = int(start_frame)

    x = np.asarray(x)
    B, L, DIM = x.shape
    assert B == 1 and DIM == num_heads * D

    allowed = _allowed(f, local_attn_size, sink_size)
    in_maps, tok_idx, T, S = _host_inputs(
        x, freqs, Wq, bq, Wk, bk, Wv, bv, Wo, bo, gq, gk,
        f, h, w, num_heads, local_attn_size, sink_size, start_frame)
    nc = build_program(num_heads, f, T, allowed)
    res = run_bass_kernel_spmd(nc, in_maps, core_ids=list(range(NC)),
                               trace=_trace)
    out = np.empty((1, L, DIM), np.float32)
    for core in range(NC):
        out[0, tok_idx[core]] = res.results[core]["out"]
    if _trace:
        kernel._last_results = res
    return out

